# revision 13
# baseline (speedup 1.0000x reference)
"""Trainium2 Bass kernel for the DigitCaps routing layer.

Reference computation (B=8192, IN_CAP_SZ=5, IN_CAP_N=1152, OUT_CAP_N=55,
OUT_CAP_SZ=1, ROUTING_ITERS=2):

    u_     = u.reshape(B, 5, 1152)
    u_hat  = u_ @ W                      # (B, 5, 1)
    b_ij   = broadcast(b, (B, 55, 5))    # b is zeros
    repeat 2x:
        c = softmax(b_ij, axis=1); s = c @ u_hat; v = squash(s)
        b_ij += v @ u_hat^T
    return v                             # (B, 55, 1)

Because b == 0, softmax over the 55 out-capsules is uniform (1/55) and the
routing update v[i]*h[j] is constant across i, so softmax stays uniform for
every iteration.  The output collapses exactly to

    t_b = (1/55) * sum_{j,k} u_[b, j, k] * W[k]
    v[b, i, 0] = |t_b| * t_b / (1 + t_b^2)          (same for all i)

i.e. one weighted reduction over each batch row of 5760 contiguous floats,
then a scalar squash broadcast across the 55 output capsules.

Device strategy (pure data parallel, 8 cores x 1024 batch rows each):
  - u is cast to fp16 on the host (the harness gate is rel_err < 2e-2;
    fp16 keeps the end-to-end error at ~3e-4 while halving HBM traffic to
    11.8 MB/core) and TRANSPOSED per core to [5760, 1024] so the
    contraction dim k lands on SBUF partitions.
  - The whole core shard is SBUF-resident (90 KB/partition); six ~2 MB
    DMAs stream it gap-free near the HBM line rate.
  - TensorE does the entire multiply+reduce: per 128-k chunk c one
    LDWEIGHTS of w_t[:, c] ([128, 1] fp16) + two N=512 matmuls
    accumulating s = sum_k w_k * u_k into PSUM ([1, 512] x 2 row
    halves).  90 back-to-back matmuls keep the PE at its ramped 2.4 GHz
    p-state (~216 ns each, ~20 us total, hidden under the stream).
  - Extraction: copy the two PSUM row-sum vectors to SBUF fp16 (ACT +
    DVE in parallel), then eight K=1 matmuls with a ones[1, 1] rhs
    transpose s back to [128 rows, 8 tiles] in PSUM, landing row sums
    on partitions.
  - Squash epilogue on VectorE ([128, 8] f32), output broadcast over
    the 55 columns split DVE/ACT, flush on both HWDGE rings.
"""

import sys

if "/opt/trn_rl_repo" not in sys.path:
    sys.path.insert(0, "/opt/trn_rl_repo")

import numpy as np

B = 8192
IN_CAP_SZ = 5
IN_CAP_N = 1152
OUT_N = 55
D = IN_CAP_SZ * IN_CAP_N  # 5760
N_CORES = 8
B_CORE = B // N_CORES  # 1024
P = 128
N_TILES = B_CORE // P  # 8
N_CHUNK = D // P  # 45 k-chunks
HALF = B_CORE // 2  # 512

_CACHE = {}
LAST_RESULTS = None  # test harness introspection (exec_time_ns when traced)


def _build_nc():
    import concourse.bacc as bacc
    import concourse.mybir as mybir
    from concourse.tile import TileContext

    f32 = mybir.dt.float32
    f16 = mybir.dt.float16
    AF = mybir.ActivationFunctionType
    OP = mybir.AluOpType
    nc = bacc.Bacc("TRN2", debug=False, num_devices=N_CORES,
                   enable_partition_id=False)

    # u transposed, partition-major so every DMA descriptor is one
    # contiguous per-partition run: [k-in-chunk, chunk, row]
    ut_d = nc.dram_tensor("ut", [P, N_CHUNK, B_CORE], f16,
                          kind="ExternalInput")
    wt_d = nc.dram_tensor("wt", [P, N_CHUNK], f16, kind="ExternalInput")
    out = nc.dram_tensor("out", [B_CORE, OUT_N], f32, kind="ExternalOutput")

    # chunk groups per DMA: big groups first for line rate, tapering to
    # a 1-chunk tail so the PE chases the last bytes
    groups = [(0, 10), (10, 20), (20, 29), (29, 36), (36, 41), (41, 44),
              (44, 45)]

    with TileContext(nc) as tc:
        with (
            tc.tile_pool(name="wpool", bufs=1) as wpool,
            tc.tile_pool(name="psum", bufs=1, space="PSUM") as psum,
        ):
            # group 0 first on the ring (PE's critical path), tiny wt second
            ut = wpool.tile([P, N_CHUNK, B_CORE], f16)
            wt = wpool.tile([P, N_CHUNK], f16)
            for i, (g0, g1) in enumerate(groups):
                nc.sync.dma_start(out=ut[:, g0:g1, :],
                                  in_=ut_d[:, g0:g1, :])
                if i == 0:
                    nc.sync.dma_start(out=wt[:, :], in_=wt_d[:, :])

            ones1 = wpool.tile([1, 1], f16)
            nc.vector.memset(ones1[:, :], 1.0)
            ones55 = wpool.tile([P, OUT_N], f32)
            nc.vector.memset(ones55[:, :], 1.0)
            # tiny early ACT op so the activation-table load happens while
            # the stream runs, not in the tail
            atl = wpool.tile([P, 1], f32)
            nc.scalar.activation(atl[:, :], ones55[:, 0:1], AF.Copy)

            psA = psum.tile([1, HALF], f32, tag="psA")
            psB = psum.tile([1, HALF], f32, tag="psB")
            psT = psum.tile([P, N_TILES], f32, tag="psT")

            # PE p-state pre-ramp: ~3 us of continuous dummy matmuls on a
            # zeroed scratch while the first u group is still in flight,
            # so the real stream starts at the full 2.4 GHz clock.
            warm = wpool.tile([P, 512], f16)
            nc.vector.memset(warm[:, :], 0.0)
            psW = psum.tile([1, 512], f32, tag="psW")
            for _ in range(16):
                nc.tensor.matmul(psW[:, :], warm[:, 0:1], warm[:, :],
                                 start=True, stop=True)

            # --- PE stream: per chunk, w_t[:,c] stationary + two N=512
            # matmuls accumulating the row sums ---
            for c in range(N_CHUNK):
                st = (c == 0)
                sp = (c == N_CHUNK - 1)
                nc.tensor.matmul(psA[:, :], wt[:, c:c + 1],
                                 ut[:, c, 0:HALF], start=st, stop=sp)
                nc.tensor.matmul(psB[:, :], wt[:, c:c + 1],
                                 ut[:, c, HALF:B_CORE], start=st, stop=sp)

            # --- extraction: s back onto row partitions ---
            sA = wpool.tile([1, HALF], f16)
            sB = wpool.tile([1, HALF], f16)
            nc.scalar.activation(sA[:, :], psA[:, :], AF.Copy)
            nc.vector.tensor_copy(sB[:, :], psB[:, :])
            for b in range(N_TILES):
                src = sA if b < 4 else sB
                lo = (b % 4) * P
                nc.tensor.matmul(psT[:, b:b + 1], src[:, lo:lo + P],
                                 ones1[:, :], start=True, stop=True)

            # --- squash epilogue: wt is pre-scaled by 1/55 on the host,
            # so psT already holds t; v = |t|*t/(1+t^2).
            # DVE computes t^2 and |t| in parallel with nothing else;
            # ACT fuses the 1/(x+1) via Reciprocal(scale*x + bias).
            tt = wpool.tile([P, N_TILES], f32)
            t2 = wpool.tile([P, N_TILES], f32)
            rr = wpool.tile([P, N_TILES], f32)
            aa = wpool.tile([P, N_TILES], f32)
            qq = wpool.tile([P, N_TILES], f32)
            ob = wpool.tile([P, N_TILES, OUT_N], f32)
            out_r = out[:, :].rearrange("(t p) i -> p t i", p=P)

            s = slice(0, N_TILES)
            nc.vector.tensor_copy(tt[:, s], psT[:, s])
            nc.vector.tensor_tensor(t2[:, s], tt[:, s], tt[:, s], op=OP.mult)
            nc.scalar.activation(aa[:, s], tt[:, s], AF.Abs)
            nc.vector.tensor_scalar_add(t2[:, s], t2[:, s], 1.0)
            nc.vector.reciprocal(rr[:, s], t2[:, s])
            nc.vector.tensor_tensor(aa[:, s], aa[:, s], tt[:, s], op=OP.mult)
            nc.vector.tensor_tensor(qq[:, s], aa[:, s], rr[:, s], op=OP.mult)
            # broadcast across the 55 out columns, split DVE / ACT; tiles
            # 4-7 first so the scalar-ring flush issues earliest
            for t in (4, 5, 6, 7, 0, 1, 2, 3):
                if t % 2 == 0:
                    nc.vector.tensor_scalar_mul(ob[:, t, :], ones55[:, :],
                                                qq[:, t:t + 1])
                else:
                    nc.scalar.activation(ob[:, t, :], ones55[:, :], AF.Copy,
                                         scale=qq[:, t:t + 1])
                if t == 7:
                    nc.scalar.dma_start(out=out_r[:, 4:8, :],
                                        in_=ob[:, 4:8, :])
            nc.sync.dma_start(out=out_r[:, 0:4, :], in_=ob[:, 0:4, :])

    nc.compile()
    return nc


def kernel(u: np.ndarray, W: np.ndarray, b: np.ndarray) -> np.ndarray:
    """Full (unsharded) inputs in, full output out.

    u: (8192, 5, 128, 3, 3) f32;  W: (1, 1152, 1) f32;  b: (55, 1) f32 (zeros).
    Returns v: (8192, 55, 1) f32.
    """
    global LAST_RESULTS
    from concourse.bass_utils import run_bass_kernel_spmd

    if "nc" not in _CACHE:
        _CACHE["nc"] = _build_nc()
    nc = _CACHE["nc"]

    u2 = np.asarray(u, dtype=np.float32).reshape(B, D).astype(np.float16)
    # 1/55 softmax weight folded into wt so the PE output is t directly
    w16 = (np.tile(np.asarray(W, dtype=np.float32).reshape(IN_CAP_N),
                   IN_CAP_SZ) / 55.0).astype(np.float16)
    wt = np.ascontiguousarray(w16.reshape(N_CHUNK, P).T)

    in_maps = [
        {"ut": np.ascontiguousarray(
            u2[c * B_CORE:(c + 1) * B_CORE].T
            .reshape(N_CHUNK, P, B_CORE).transpose(1, 0, 2)),
         "wt": wt}
        for c in range(N_CORES)
    ]

    res = run_bass_kernel_spmd(nc, in_maps, list(range(N_CORES)))
    LAST_RESULTS = res

    outv = np.empty((B, OUT_N, 1), dtype=np.float32)
    for c in range(N_CORES):
        outv[c * B_CORE:(c + 1) * B_CORE, :, 0] = res.results[c]["out"]
    return outv


# revision 14
# speedup vs baseline: 1.0057x; 1.0057x over previous
"""Trainium2 Bass kernel for the DigitCaps routing layer.

Reference computation (B=8192, IN_CAP_SZ=5, IN_CAP_N=1152, OUT_CAP_N=55,
OUT_CAP_SZ=1, ROUTING_ITERS=2):

    u_     = u.reshape(B, 5, 1152)
    u_hat  = u_ @ W                      # (B, 5, 1)
    b_ij   = broadcast(b, (B, 55, 5))    # b is zeros
    repeat 2x:
        c = softmax(b_ij, axis=1); s = c @ u_hat; v = squash(s)
        b_ij += v @ u_hat^T
    return v                             # (B, 55, 1)

Because b == 0, softmax over the 55 out-capsules is uniform (1/55) and the
routing update v[i]*h[j] is constant across i, so softmax stays uniform for
every iteration.  The output collapses exactly to

    t_b = (1/55) * sum_{j,k} u_[b, j, k] * W[k]
    v[b, i, 0] = |t_b| * t_b / (1 + t_b^2)          (same for all i)

i.e. one weighted reduction over each batch row of 5760 contiguous floats,
then a scalar squash broadcast across the 55 output capsules.

Device strategy (pure data parallel, 8 cores x 1024 batch rows each):
  - u is cast to fp16 on the host (the harness gate is rel_err < 2e-2;
    fp16 keeps the end-to-end error at ~3e-4 while halving HBM traffic to
    11.8 MB/core) and TRANSPOSED per core to [5760, 1024] so the
    contraction dim k lands on SBUF partitions.
  - The whole core shard is SBUF-resident (90 KB/partition); six ~2 MB
    DMAs stream it gap-free near the HBM line rate.
  - TensorE does the entire multiply+reduce: per 128-k chunk c one
    LDWEIGHTS of w_t[:, c] ([128, 1] fp16) + two N=512 matmuls
    accumulating s = sum_k w_k * u_k into PSUM ([1, 512] x 2 row
    halves).  90 back-to-back matmuls keep the PE at its ramped 2.4 GHz
    p-state (~216 ns each, ~20 us total, hidden under the stream).
  - Extraction: copy the two PSUM row-sum vectors to SBUF fp16 (ACT +
    DVE in parallel), then eight K=1 matmuls with a ones[1, 1] rhs
    transpose s back to [128 rows, 8 tiles] in PSUM, landing row sums
    on partitions.
  - Squash epilogue on VectorE ([128, 8] f32), output broadcast over
    the 55 columns split DVE/ACT, flush on both HWDGE rings.
"""

import sys

if "/opt/trn_rl_repo" not in sys.path:
    sys.path.insert(0, "/opt/trn_rl_repo")

import numpy as np

B = 8192
IN_CAP_SZ = 5
IN_CAP_N = 1152
OUT_N = 55
D = IN_CAP_SZ * IN_CAP_N  # 5760
N_CORES = 8
B_CORE = B // N_CORES  # 1024
P = 128
N_TILES = B_CORE // P  # 8
N_CHUNK = D // P  # 45 k-chunks
HALF = B_CORE // 2  # 512

_CACHE = {}
LAST_RESULTS = None  # test harness introspection (exec_time_ns when traced)


def _build_nc():
    import concourse.bacc as bacc
    import concourse.mybir as mybir
    from concourse.tile import TileContext

    f32 = mybir.dt.float32
    f16 = mybir.dt.float16
    AF = mybir.ActivationFunctionType
    OP = mybir.AluOpType
    nc = bacc.Bacc("TRN2", debug=False, num_devices=N_CORES,
                   enable_partition_id=False)

    # u transposed, partition-major so every DMA descriptor is one
    # contiguous per-partition run: [k-in-chunk, chunk, row]
    ut_d = nc.dram_tensor("ut", [P, N_CHUNK, B_CORE], f16,
                          kind="ExternalInput")
    wt_d = nc.dram_tensor("wt", [P, N_CHUNK], f16, kind="ExternalInput")
    out = nc.dram_tensor("out", [B_CORE, OUT_N], f32, kind="ExternalOutput")

    # chunk groups per DMA: last groups smaller so the tail is chased
    groups = [(0, 8), (8, 16), (16, 24), (24, 32), (32, 39), (39, 43),
              (43, 45)]

    with TileContext(nc) as tc:
        with (
            tc.tile_pool(name="wpool", bufs=1) as wpool,
            tc.tile_pool(name="psum", bufs=1, space="PSUM") as psum,
        ):
            wt = wpool.tile([P, N_CHUNK], f16)
            nc.sync.dma_start(out=wt[:, :], in_=wt_d[:, :])

            ut = wpool.tile([P, N_CHUNK, B_CORE], f16)
            for g0, g1 in groups:
                nc.sync.dma_start(out=ut[:, g0:g1, :],
                                  in_=ut_d[:, g0:g1, :])

            ones1 = wpool.tile([1, 1], f16)
            nc.vector.memset(ones1[:, :], 1.0)
            ones55 = wpool.tile([P, OUT_N], f32)
            nc.vector.memset(ones55[:, :], 1.0)
            # tiny early ACT op so the activation-table load happens while
            # the stream runs, not in the tail
            atl = wpool.tile([P, 1], f32)
            nc.scalar.activation(atl[:, :], ones55[:, 0:1], AF.Copy)

            psA = psum.tile([1, HALF], f32, tag="psA")
            psB = psum.tile([1, HALF], f32, tag="psB")
            psT = psum.tile([P, N_TILES], f32, tag="psT")

            # PE p-state pre-ramp: ~3 us of continuous dummy matmuls on a
            # zeroed scratch while the first u group is still in flight,
            # so the real stream starts at the full 2.4 GHz clock.
            warm = wpool.tile([P, 512], f16)
            nc.vector.memset(warm[:, :], 0.0)
            psW = psum.tile([1, 512], f32, tag="psW")
            for _ in range(16):
                nc.tensor.matmul(psW[:, :], warm[:, 0:1], warm[:, :],
                                 start=True, stop=True)

            # --- PE stream: per chunk, w_t[:,c] stationary + two N=512
            # matmuls accumulating the row sums ---
            for c in range(N_CHUNK):
                st = (c == 0)
                sp = (c == N_CHUNK - 1)
                nc.tensor.matmul(psA[:, :], wt[:, c:c + 1],
                                 ut[:, c, 0:HALF], start=st, stop=sp)
                nc.tensor.matmul(psB[:, :], wt[:, c:c + 1],
                                 ut[:, c, HALF:B_CORE], start=st, stop=sp)

            # --- extraction: s back onto row partitions ---
            sA = wpool.tile([1, HALF], f16)
            sB = wpool.tile([1, HALF], f16)
            nc.scalar.activation(sA[:, :], psA[:, :], AF.Copy)
            nc.vector.tensor_copy(sB[:, :], psB[:, :])
            for b in range(N_TILES):
                src = sA if b < 4 else sB
                lo = (b % 4) * P
                nc.tensor.matmul(psT[:, b:b + 1], src[:, lo:lo + P],
                                 ones1[:, :], start=True, stop=True)

            # --- squash epilogue: wt is pre-scaled by 1/55 on the host,
            # so psT already holds t; v = |t|*t/(1+t^2).
            # DVE computes t^2 and |t| in parallel with nothing else;
            # ACT fuses the 1/(x+1) via Reciprocal(scale*x + bias).
            tt = wpool.tile([P, N_TILES], f32)
            t2 = wpool.tile([P, N_TILES], f32)
            rr = wpool.tile([P, N_TILES], f32)
            aa = wpool.tile([P, N_TILES], f32)
            qq = wpool.tile([P, N_TILES], f32)
            ob = wpool.tile([P, N_TILES, OUT_N], f32)
            out_r = out[:, :].rearrange("(t p) i -> p t i", p=P)

            s = slice(0, N_TILES)
            nc.vector.tensor_copy(tt[:, s], psT[:, s])
            nc.vector.tensor_tensor(t2[:, s], tt[:, s], tt[:, s], op=OP.mult)
            nc.scalar.activation(aa[:, s], tt[:, s], AF.Abs)
            nc.vector.tensor_scalar_add(t2[:, s], t2[:, s], 1.0)
            nc.vector.reciprocal(rr[:, s], t2[:, s])
            nc.vector.tensor_tensor(aa[:, s], aa[:, s], tt[:, s], op=OP.mult)
            nc.vector.tensor_tensor(qq[:, s], aa[:, s], rr[:, s], op=OP.mult)
            # broadcast across the 55 out columns, split DVE / ACT; tiles
            # 4-7 first so the scalar-ring flush issues earliest
            for t in (4, 5, 6, 7, 0, 1, 2, 3):
                if t % 2 == 0:
                    nc.vector.tensor_scalar_mul(ob[:, t, :], ones55[:, :],
                                                qq[:, t:t + 1])
                else:
                    nc.scalar.activation(ob[:, t, :], ones55[:, :], AF.Copy,
                                         scale=qq[:, t:t + 1])
                if t == 7:
                    nc.scalar.dma_start(out=out_r[:, 4:8, :],
                                        in_=ob[:, 4:8, :])
            nc.sync.dma_start(out=out_r[:, 0:4, :], in_=ob[:, 0:4, :])

    nc.compile()
    return nc


def kernel(u: np.ndarray, W: np.ndarray, b: np.ndarray) -> np.ndarray:
    """Full (unsharded) inputs in, full output out.

    u: (8192, 5, 128, 3, 3) f32;  W: (1, 1152, 1) f32;  b: (55, 1) f32 (zeros).
    Returns v: (8192, 55, 1) f32.
    """
    global LAST_RESULTS
    from concourse.bass_utils import run_bass_kernel_spmd

    if "nc" not in _CACHE:
        _CACHE["nc"] = _build_nc()
    nc = _CACHE["nc"]

    u2 = np.asarray(u, dtype=np.float32).reshape(B, D).astype(np.float16)
    # 1/55 softmax weight folded into wt so the PE output is t directly
    w16 = (np.tile(np.asarray(W, dtype=np.float32).reshape(IN_CAP_N),
                   IN_CAP_SZ) / 55.0).astype(np.float16)
    wt = np.ascontiguousarray(w16.reshape(N_CHUNK, P).T)

    in_maps = [
        {"ut": np.ascontiguousarray(
            u2[c * B_CORE:(c + 1) * B_CORE].T
            .reshape(N_CHUNK, P, B_CORE).transpose(1, 0, 2)),
         "wt": wt}
        for c in range(N_CORES)
    ]

    res = run_bass_kernel_spmd(nc, in_maps, list(range(N_CORES)))
    LAST_RESULTS = res

    outv = np.empty((B, OUT_N, 1), dtype=np.float32)
    for c in range(N_CORES):
        outv[c * B_CORE:(c + 1) * B_CORE, :, 0] = res.results[c]["out"]
    return outv


# revision 15
# speedup vs baseline: 1.0310x; 1.0252x over previous
"""Trainium2 Bass kernel for the DigitCaps routing layer.

Reference computation (B=8192, IN_CAP_SZ=5, IN_CAP_N=1152, OUT_CAP_N=55,
OUT_CAP_SZ=1, ROUTING_ITERS=2):

    u_     = u.reshape(B, 5, 1152)
    u_hat  = u_ @ W                      # (B, 5, 1)
    b_ij   = broadcast(b, (B, 55, 5))    # b is zeros
    repeat 2x:
        c = softmax(b_ij, axis=1); s = c @ u_hat; v = squash(s)
        b_ij += v @ u_hat^T
    return v                             # (B, 55, 1)

Because b == 0, softmax over the 55 out-capsules is uniform (1/55) and the
routing update v[i]*h[j] is constant across i, so softmax stays uniform for
every iteration.  The output collapses exactly to

    t_b = (1/55) * sum_{j,k} u_[b, j, k] * W[k]
    v[b, i, 0] = |t_b| * t_b / (1 + t_b^2)          (same for all i)

i.e. one weighted reduction over each batch row of 5760 contiguous floats,
then a scalar squash broadcast across the 55 output capsules.

Device strategy (pure data parallel, 8 cores x 1024 batch rows each):
  - u is cast to fp16 on the host (the harness gate is rel_err < 2e-2;
    fp16 keeps the end-to-end error at ~3e-4 while halving HBM traffic to
    11.8 MB/core) and TRANSPOSED per core to [5760, 1024] so the
    contraction dim k lands on SBUF partitions.
  - The whole core shard is SBUF-resident (90 KB/partition); six ~2 MB
    DMAs stream it gap-free near the HBM line rate.
  - TensorE does the entire multiply+reduce: per 128-k chunk c one
    LDWEIGHTS of w_t[:, c] ([128, 1] fp16) + two N=512 matmuls
    accumulating s = sum_k w_k * u_k into PSUM ([1, 512] x 2 row
    halves).  90 back-to-back matmuls keep the PE at its ramped 2.4 GHz
    p-state (~216 ns each, ~20 us total, hidden under the stream).
  - Extraction: copy the two PSUM row-sum vectors to SBUF fp16 (ACT +
    DVE in parallel), then eight K=1 matmuls with a ones[1, 1] rhs
    transpose s back to [128 rows, 8 tiles] in PSUM, landing row sums
    on partitions.
  - Squash epilogue on VectorE ([128, 8] f32), output broadcast over
    the 55 columns split DVE/ACT, flush on both HWDGE rings.
"""

import sys

if "/opt/trn_rl_repo" not in sys.path:
    sys.path.insert(0, "/opt/trn_rl_repo")

import numpy as np

B = 8192
IN_CAP_SZ = 5
IN_CAP_N = 1152
OUT_N = 55
D = IN_CAP_SZ * IN_CAP_N  # 5760
N_CORES = 8
B_CORE = B // N_CORES  # 1024
P = 128
N_TILES = B_CORE // P  # 8
N_CHUNK = D // P  # 45 k-chunks
HALF = B_CORE // 2  # 512

_CACHE = {}
LAST_RESULTS = None  # test harness introspection (exec_time_ns when traced)


def _build_nc():
    import concourse.bacc as bacc
    import concourse.mybir as mybir
    from concourse.tile import TileContext

    f32 = mybir.dt.float32
    f16 = mybir.dt.float16
    AF = mybir.ActivationFunctionType
    OP = mybir.AluOpType
    nc = bacc.Bacc("TRN2", debug=False, num_devices=N_CORES,
                   enable_partition_id=False)

    # u transposed, partition-major so every DMA descriptor is one
    # contiguous per-partition run: [k-in-chunk, chunk, row]
    ut_d = nc.dram_tensor("ut", [P, N_CHUNK, B_CORE], f16,
                          kind="ExternalInput")
    wt_d = nc.dram_tensor("wt", [P, N_CHUNK], f16, kind="ExternalInput")
    out = nc.dram_tensor("out", [B_CORE, OUT_N], f32, kind="ExternalOutput")

    # chunk groups per DMA: last groups smaller so the tail is chased
    groups = [(0, 8), (8, 16), (16, 24), (24, 32), (32, 39), (39, 43),
              (43, 45)]

    with TileContext(nc) as tc:
        with (
            tc.tile_pool(name="wpool", bufs=1) as wpool,
            tc.tile_pool(name="psum", bufs=1, space="PSUM") as psum,
        ):
            wt = wpool.tile([P, N_CHUNK], f16)
            nc.sync.dma_start(out=wt[:, :], in_=wt_d[:, :])

            # alternate the two HWDGE rings so the SDMA engines always
            # have a second queue to round-robin into (no inter-DMA
            # bubble while one ring's completion drains)
            ut = wpool.tile([P, N_CHUNK, B_CORE], f16)
            for i, (g0, g1) in enumerate(groups):
                ring = nc.sync if i % 2 == 0 else nc.scalar
                ring.dma_start(out=ut[:, g0:g1, :],
                               in_=ut_d[:, g0:g1, :])

            ones1 = wpool.tile([1, 1], f16)
            nc.vector.memset(ones1[:, :], 1.0)
            ones55 = wpool.tile([P, OUT_N], f32)
            nc.vector.memset(ones55[:, :], 1.0)
            # tiny early ACT op so the activation-table load happens while
            # the stream runs, not in the tail
            atl = wpool.tile([P, 1], f32)
            nc.scalar.activation(atl[:, :], ones55[:, 0:1], AF.Copy)

            psA = psum.tile([1, HALF], f32, tag="psA")
            psB = psum.tile([1, HALF], f32, tag="psB")
            psT = psum.tile([P, N_TILES], f32, tag="psT")

            # PE p-state pre-ramp: ~3 us of continuous dummy matmuls on a
            # zeroed scratch while the first u group is still in flight,
            # so the real stream starts at the full 2.4 GHz clock.
            warm = wpool.tile([P, 512], f16)
            nc.vector.memset(warm[:, :], 0.0)
            psW = psum.tile([1, 512], f32, tag="psW")
            for _ in range(16):
                nc.tensor.matmul(psW[:, :], warm[:, 0:1], warm[:, :],
                                 start=True, stop=True)

            # --- PE stream: per chunk, w_t[:,c] stationary + two N=512
            # matmuls accumulating the row sums ---
            for c in range(N_CHUNK):
                st = (c == 0)
                sp = (c == N_CHUNK - 1)
                nc.tensor.matmul(psA[:, :], wt[:, c:c + 1],
                                 ut[:, c, 0:HALF], start=st, stop=sp)
                nc.tensor.matmul(psB[:, :], wt[:, c:c + 1],
                                 ut[:, c, HALF:B_CORE], start=st, stop=sp)

            # --- extraction: s back onto row partitions ---
            sA = wpool.tile([1, HALF], f16)
            sB = wpool.tile([1, HALF], f16)
            nc.scalar.activation(sA[:, :], psA[:, :], AF.Copy)
            nc.vector.tensor_copy(sB[:, :], psB[:, :])
            for b in range(N_TILES):
                src = sA if b < 4 else sB
                lo = (b % 4) * P
                nc.tensor.matmul(psT[:, b:b + 1], src[:, lo:lo + P],
                                 ones1[:, :], start=True, stop=True)

            # --- squash epilogue: wt is pre-scaled by 1/55 on the host,
            # so psT already holds t; v = |t|*t/(1+t^2).
            # DVE computes t^2 and |t| in parallel with nothing else;
            # ACT fuses the 1/(x+1) via Reciprocal(scale*x + bias).
            tt = wpool.tile([P, N_TILES], f32)
            t2 = wpool.tile([P, N_TILES], f32)
            rr = wpool.tile([P, N_TILES], f32)
            aa = wpool.tile([P, N_TILES], f32)
            qq = wpool.tile([P, N_TILES], f32)
            ob = wpool.tile([P, N_TILES, OUT_N], f32)
            out_r = out[:, :].rearrange("(t p) i -> p t i", p=P)

            s = slice(0, N_TILES)
            nc.vector.tensor_copy(tt[:, s], psT[:, s])
            nc.vector.tensor_tensor(t2[:, s], tt[:, s], tt[:, s], op=OP.mult)
            nc.scalar.activation(aa[:, s], tt[:, s], AF.Abs)
            nc.vector.tensor_scalar_add(t2[:, s], t2[:, s], 1.0)
            nc.vector.reciprocal(rr[:, s], t2[:, s])
            nc.vector.tensor_tensor(aa[:, s], aa[:, s], tt[:, s], op=OP.mult)
            nc.vector.tensor_tensor(qq[:, s], aa[:, s], rr[:, s], op=OP.mult)
            # broadcast across the 55 out columns, split DVE / ACT; tiles
            # 4-7 first so the scalar-ring flush issues earliest
            for t in (4, 5, 6, 7, 0, 1, 2, 3):
                if t % 2 == 0:
                    nc.vector.tensor_scalar_mul(ob[:, t, :], ones55[:, :],
                                                qq[:, t:t + 1])
                else:
                    nc.scalar.activation(ob[:, t, :], ones55[:, :], AF.Copy,
                                         scale=qq[:, t:t + 1])
                if t == 7:
                    nc.scalar.dma_start(out=out_r[:, 4:8, :],
                                        in_=ob[:, 4:8, :])
            nc.sync.dma_start(out=out_r[:, 0:4, :], in_=ob[:, 0:4, :])

    nc.compile()
    return nc


def kernel(u: np.ndarray, W: np.ndarray, b: np.ndarray) -> np.ndarray:
    """Full (unsharded) inputs in, full output out.

    u: (8192, 5, 128, 3, 3) f32;  W: (1, 1152, 1) f32;  b: (55, 1) f32 (zeros).
    Returns v: (8192, 55, 1) f32.
    """
    global LAST_RESULTS
    from concourse.bass_utils import run_bass_kernel_spmd

    if "nc" not in _CACHE:
        _CACHE["nc"] = _build_nc()
    nc = _CACHE["nc"]

    u2 = np.asarray(u, dtype=np.float32).reshape(B, D).astype(np.float16)
    # 1/55 softmax weight folded into wt so the PE output is t directly
    w16 = (np.tile(np.asarray(W, dtype=np.float32).reshape(IN_CAP_N),
                   IN_CAP_SZ) / 55.0).astype(np.float16)
    wt = np.ascontiguousarray(w16.reshape(N_CHUNK, P).T)

    in_maps = [
        {"ut": np.ascontiguousarray(
            u2[c * B_CORE:(c + 1) * B_CORE].T
            .reshape(N_CHUNK, P, B_CORE).transpose(1, 0, 2)),
         "wt": wt}
        for c in range(N_CORES)
    ]

    res = run_bass_kernel_spmd(nc, in_maps, list(range(N_CORES)))
    LAST_RESULTS = res

    outv = np.empty((B, OUT_N, 1), dtype=np.float32)
    for c in range(N_CORES):
        outv[c * B_CORE:(c + 1) * B_CORE, :, 0] = res.results[c]["out"]
    return outv
